# revision 21
# baseline (speedup 1.0000x reference)
"""Trainium2 Bass kernel for nn_Compressor (NSA-style windowed KV compression).

Math (per reference):
  kv   = x @ wkv_w.T                     [B, S, 1024]
  gate = sigmoid(x @ wgate_w.T)
  kv   = kv * gate + tile(ape)           (ape per position-within-window)
  kv   = mean over windows of 4          [B, S/4, 2, 512]
  out  = norm_w * kv * rsqrt(mean(kv^2, -1) + eps)   [B, S/2, 512]

Distribution: x flattened to [B*S, 4096] = [16384, 4096], sharded into 8
contiguous 2048-row blocks (whole windows per shard); weights replicated.
Each core computes its [1024, 512] output shard; host concatenates.

On-chip strategy (per core):
  The PE is moving-row-rate bound (~1 moving row/cycle regardless of
  dtype; fp8 DoubleRow packs 2 contraction rows per partition, i.e. 2x K
  per moving row, but does not speed up the rows themselves). Moving-row
  budget per core: a full K=4096 pass over 1024 output cols for 16
  s-tiles costs 512K rows in fp16, 256K rows in fp8-DR.

  - kv needs near-fp16 accuracy (no attenuation downstream): computed in
    fp16 (512K rows). An fp8-DR hi/lo-split version needs 3 passes (the
    2e-2 rel-err budget does not survive unsplit operands) = 768K rows,
    strictly worse — so fp16.
  - gate pre-activations tolerate fp8 noise (the error enters the output
    through kv * sigmoid'(t), a ~0.34x attenuation): single fp8-DR pass
    over e4m3 x and e4m3 weights (256K rows). Weights are pre-scaled by
    WS=64 on the host so e4m3 normals cover them; the descale folds into
    the sigmoid's input scale.
  Total 768K rows = 0.75x of an all-fp16 kernel. Measured end-to-end
  relnorm ~1.4e-2 on the fixed seed-0 inputs (budget 2e-2).

  Pipeline notes:
  - epilogue out-DMAs issue on the ACT HWDGE queue: on the SP queue they
    sit behind the next x-prefetch's WAR wait (SP blocks in-order), which
    stalls the small-pool recycle -> epilogue -> PSUM drain -> PE.
  - x tiles for block N+1 are prefetched before block N's matmuls are
    emitted so SP reaches the DMA early.
  - ape is added after pooling (pooling is linear; mean_r ape[r] is one
    constant row per coff stream), fused into the PSUM->SBUF drain on the
    DVE. The final normalize is split: norm_w multiply on the (otherwise
    idle) Pool engine, rinv scale on ACT.
"""

import sys

sys.path.insert(0, "/opt/trn_rl_repo")

import numpy as np
import ml_dtypes

import concourse.tile as tile
from concourse import bacc, mybir
from concourse.bass_utils import run_bass_kernel_spmd

HALF = np.float16
E4 = ml_dtypes.float8_e4m3

N_CORES = 8
B, S, D = 4, 4096, 4096
R = 4                  # compress ratio (window)
HD = 512               # head dim
OD = 1024              # coff * head_dim
EPS = 1e-6
WS = 64.0              # host-side gate-weight prescale (entries ~N(0,1/64^2))

ROWS = (B * S) // N_CORES      # 2048 sequence rows per core
DC = D // 128                  # 32 fp16 contraction chunks (kv path)
DC2 = D // 256                 # 16 fp8-DR contraction chunks (gate path)
NT = ROWS // 128               # 16 s-tiles per core
SBLK = 256                     # x columns loaded per DMA block (2 s-tiles)
NW_TILE = 128 // R             # 32 windows per s-tile
DR = mybir.MatmulPerfMode.DoubleRow

_CACHED_NC = None


def _build_nc(reps=1, skip_epi=False, static_x=False):
    nc = bacc.Bacc("TRN2", target_bir_lowering=False, debug=False,
                   num_devices=N_CORES)
    f32 = mybir.dt.float32
    f16 = mybir.dt.float16
    f8 = mybir.dt.float8e4

    # x fp16 for kv: [DC, 128, ROWS], logical k = dc*128 + p
    x16 = nc.dram_tensor("x16", [DC, 128, ROWS], f16, kind="ExternalInput").ap()
    # x e4m3 for gate: [DC2, 2, 128, ROWS], logical k = dc*256 + slot*128 + p
    xh = nc.dram_tensor("xh", [DC2, 2, 128, ROWS], f8, kind="ExternalInput").ap()
    wkv = nc.dram_tensor("wkv", [DC, 128, OD], f16, kind="ExternalInput").ap()
    wgh = nc.dram_tensor("wgh", [DC2, 2, 128, OD], f8, kind="ExternalInput").ap()
    apep = nc.dram_tensor("apep", [NW_TILE, OD], f32, kind="ExternalInput").ap()
    nrmb = nc.dram_tensor("nrmb", [NW_TILE, HD], f32, kind="ExternalInput").ap()
    poolm = nc.dram_tensor("poolm", [128, NW_TILE], f16, kind="ExternalInput").ap()
    out = nc.dram_tensor("out", [ROWS // R * 2, HD], f32, kind="ExternalOutput").ap()

    x16_v = x16.rearrange("dc p s -> p dc s")
    xh_v = xh.rearrange("dc two p s -> p dc two s")
    wkv_v = wkv.rearrange("dc p o -> p dc o")
    wgh_v = wgh.rearrange("dc two p o -> p dc two o")
    out_v = out.rearrange("(w two) h -> w two h", two=2)

    with tile.TileContext(nc) as tc:
        with (
            tc.tile_pool(name="const", bufs=1) as const_pool,
            tc.tile_pool(name="wpool", bufs=1) as wpool,
            tc.tile_pool(name="xpool", bufs=2) as xpool,
            tc.tile_pool(name="acts", bufs=2) as acts,
            tc.tile_pool(name="small", bufs=2) as small,
            tc.tile_pool(name="mm", bufs=3, space="PSUM") as psum_pool,
            tc.tile_pool(name="pl", bufs=2, space="PSUM") as pool_psum,
        ):
            WSL = 4   # fp16 dc chunks per kv-weight DMA slice (512 K values)

            wkv_sl, wgh_sl = [], []
            for s0 in range(DC // WSL):
                t = wpool.tile([128, WSL, OD], f16, tag=f"wkv{s0}")
                nc.sync.dma_start(t[:], wkv_v[:, s0 * WSL:(s0 + 1) * WSL, :])
                wkv_sl.append(t)
                t = wpool.tile([128, WSL // 2, 2, OD], f8, tag=f"wgh{s0}")
                nc.sync.dma_start(
                    t[:], wgh_v[:, s0 * WSL // 2:(s0 + 1) * WSL // 2, :, :])
                wgh_sl.append(t)

            apep_sb = const_pool.tile([NW_TILE, OD], f32)
            nc.sync.dma_start(apep_sb[:], apep)
            nrmb_sb = const_pool.tile([NW_TILE, HD], f32)
            nc.sync.dma_start(nrmb_sb[:], nrmb)
            poolm_sb = const_pool.tile([128, NW_TILE], f16)
            nc.sync.dma_start(poolm_sb[:], poolm)
            eps_sb = const_pool.tile([NW_TILE, 1], f32)
            nc.gpsimd.memset(eps_sb[:], EPS)

            def load_xblk(blk):
                t16 = xpool.tile([128, DC, SBLK], f16, tag="x16")
                nc.sync.dma_start(
                    t16[:], x16_v[:, :, blk * SBLK:(blk + 1) * SBLK])
                t8 = xpool.tile([128, DC2, 2, SBLK], f8, tag="xh")
                nc.sync.dma_start(
                    t8[:], xh_v[:, :, :, blk * SBLK:(blk + 1) * SBLK])
                return t16, t8

            def epilogue(ps_kv, ps_g, i, c):
                cs = slice(c * HD, (c + 1) * HD)
                gate_sb = acts.tile([128, HD], f32, tag="gate")
                nc.scalar.activation(gate_sb[:], ps_g[:],
                                     mybir.ActivationFunctionType.Sigmoid,
                                     scale=1.0 / WS)
                kvg16 = acts.tile([128, HD], f16, tag="kvg16")
                nc.vector.tensor_mul(kvg16[:], ps_kv[:], gate_sb[:])
                pooled_ps = pool_psum.tile([NW_TILE, HD], f32, tag="pooled")
                nc.tensor.matmul(pooled_ps[:], poolm_sb[:], kvg16[:],
                                 start=True, stop=True)
                # pooled += mean-of-window ape (constant row), fused with the
                # PSUM -> SBUF drain (GPSIMD can't read PSUM on TRN2)
                pooled_sb = small.tile([NW_TILE, HD], f32, tag="pooled_sb")
                nc.vector.tensor_add(pooled_sb[:], pooled_ps[:], apep_sb[:, cs])
                # RMSNorm over the free (head) dim
                sqj = small.tile([NW_TILE, HD], f32, tag="sqj")
                ssq = small.tile([NW_TILE, 1], f32, tag="ssq")
                nc.vector.tensor_mul(sqj[:], pooled_sb[:], pooled_sb[:])
                nc.vector.reduce_sum(ssq[:], sqj[:], axis=mybir.AxisListType.X)
                std = small.tile([NW_TILE, 1], f32, tag="std")
                nc.scalar.activation(std[:], ssq[:],
                                     mybir.ActivationFunctionType.Sqrt,
                                     bias=eps_sb[:], scale=1.0 / HD)
                rinv = small.tile([NW_TILE, 1], f32, tag="rinv")
                nc.vector.reciprocal(rinv[:], std[:])
                pn = small.tile([NW_TILE, HD], f32, tag="pn")
                nc.gpsimd.tensor_mul(pn[:], pooled_sb[:], nrmb_sb[:])
                onorm = small.tile([NW_TILE, HD], f32, tag="onorm")
                nc.scalar.mul(onorm[:], pn[:], rinv[:])
                # out-DMA on the ACT HWDGE queue: its wait (onorm ready) is
                # satisfied the moment ACT reaches it, so it never blocks.
                nc.scalar.dma_start(
                    out_v[i * NW_TILE:(i + 1) * NW_TILE, c, :], onorm[:])

            pending = []

            def flush(keep):
                while len(pending) > keep:
                    args = pending.pop(0)
                    if not skip_epi or (keep == 0 and not pending):
                        epilogue(*args)

            NBLK = NT * 128 // SBLK
            xt0 = None
            nxt = None
            for _rep in range(reps):
              for blk in range(NBLK):
                  if static_x:
                      if xt0 is None:
                          xt0 = load_xblk(0)
                      xt16, xt8 = xt0
                  else:
                      # prefetch: this block's tiles were issued one block ago
                      if nxt is None:
                          nxt = load_xblk(0)
                      xt16, xt8 = nxt
                      if not (_rep == reps - 1 and blk == NBLK - 1):
                          nxt = load_xblk((blk + 1) % NBLK)
                  for j in range(SBLK // 128):
                      for c in range(2):
                          i = blk * (SBLK // 128) + j
                          ps_kv = psum_pool.tile([128, HD], f32, tag="ps_kv")
                          ps_g = psum_pool.tile([128, HD], f32, tag="ps_g")
                          js = slice(j * 128, (j + 1) * 128)
                          ws_ = slice(c * HD, (c + 1) * HD)
                          # gate first so its epilogue (sigmoid) can start
                          # while the kv matmuls still run on the PE.
                          # (Both finer interleaving of gate/kv chunks and
                          # coarser batching of both gate chunks per s-tile
                          # measured ~5-8% slower.)
                          for dc in range(DC2):
                              nc.tensor.matmul(
                                  ps_g[:], xt8[:, dc, :, js],
                                  wgh_sl[dc // (WSL // 2)]
                                        [:, dc % (WSL // 2), :, ws_],
                                  start=(dc == 0), stop=(dc == DC2 - 1),
                                  perf_mode=DR)
                          for dc in range(DC):
                              nc.tensor.matmul(
                                  ps_kv[:], xt16[:, dc, js],
                                  wkv_sl[dc // WSL][:, dc % WSL, ws_],
                                  start=(dc == 0), stop=(dc == DC - 1))
                          pending.append((ps_kv, ps_g, i, c))
                          flush(1)
            flush(0)

    nc.compile()
    return nc


def _get_nc():
    global _CACHED_NC
    if _CACHED_NC is None:
        _CACHED_NC = _build_nc()
    return _CACHED_NC


def _prep_in_maps(x, wkv_w, wgate_w, ape, norm_w):
    x = np.asarray(x, dtype=np.float32)
    wkv_w = np.asarray(wkv_w, dtype=np.float32)
    wgate_w = np.asarray(wgate_w, dtype=np.float32)
    ape = np.asarray(ape, dtype=np.float32)
    norm_w = np.asarray(norm_w, dtype=np.float32)

    xt = np.ascontiguousarray(x.reshape(B * S, D).T)          # [D, B*S]
    xt16_full = xt.astype(HALF)
    xh_full = xt.astype(E4)

    wkv16 = np.ascontiguousarray(wkv_w.T.astype(HALF)).reshape(DC, 128, OD)
    wgh = np.ascontiguousarray(wgate_w.T * WS).astype(E4).reshape(
        DC2, 2, 128, OD)

    apep = np.ascontiguousarray(
        np.tile(ape.mean(axis=0, dtype=np.float64).astype(np.float32)[None, :],
                (NW_TILE, 1)))                                 # [32, OD]
    nrmb = np.ascontiguousarray(np.tile(norm_w[None, :], (NW_TILE, 1)))
    poolm = np.zeros((128, NW_TILE), np.float32)
    poolm[np.arange(128), np.arange(128) // R] = 1.0 / R
    poolm = poolm.astype(HALF)

    in_maps = []
    for k in range(N_CORES):
        cs = slice(k * ROWS, (k + 1) * ROWS)
        x16_k = np.ascontiguousarray(xt16_full[:, cs]).reshape(DC, 128, ROWS)
        xh_k = np.ascontiguousarray(xh_full[:, cs]).reshape(DC2, 2, 128, ROWS)
        in_maps.append({
            "x16": x16_k, "xh": xh_k, "wkv": wkv16, "wgh": wgh,
            "apep": apep, "nrmb": nrmb, "poolm": poolm,
        })
    return in_maps


def kernel(x, wkv_w, wgate_w, ape, norm_w):
    nc = _get_nc()
    in_maps = _prep_in_maps(x, wkv_w, wgate_w, ape, norm_w)
    try:
        res = run_bass_kernel_spmd(nc, in_maps, list(range(N_CORES)))
    except Exception:
        # Transient axon-transport failures are retryable; a wedged device
        # (NRT_EXEC_UNIT_UNRECOVERABLE) recovers with a fresh PJRT session.
        try:
            import jax
            jax.clear_backends()
        except Exception:
            pass
        res = run_bass_kernel_spmd(nc, in_maps, list(range(N_CORES)))
    shards = [res.results[k]["out"] for k in range(N_CORES)]
    return np.concatenate(shards, axis=0).reshape(B, S // R * 2, HD)
